# revision 52
# baseline (speedup 1.0000x reference)
"""Trainium2 Bass kernel for batched bilinear (general) attention.

Reference computation (all fp32):
    psi = einsum("bth,ah->bta", h_enc, W_psi) + b_psi        # [B, T, A]
    phi = einsum("qbh,ah->qba", h_dec, W_phi) + b_phi        # [Q, B, A]
    e   = einsum("bta,qba->btq", psi, phi)                   # [B, T, Q]
    a   = softmax(e, axis=1)                                 # over T
    c   = einsum("bth,btq->bqh", h_enc, a)                   # [B, Q, H]

Algebraic refactor: e[b,t,q] = enc_t . M . dec_q + enc_t . u + (per-q const)
with M = W_psi^T @ W_phi [H,H], u = W_psi^T @ b_phi.  Per-q-column constants
are softmax-invariant and dropped.  The host folds weights into
Z[b] = M @ dec_b^T + u [H, Q] (tiny); the device computes e = enc @ Z,
softmax over T, and c = p^T @ enc.

Precision: z = f16(Z) single-channel by default (ZDUAL=1 packs a zl
correction channel alongside).  enc streams in f16.  The exps read the e
psum directly (per-chunk max from the same psum), so no eT assembly is
needed; measured end-to-end rel-err ~8e-3 vs the 2e-2 gate.

Schedule (per core, BL=2 batches: b0 "chip", b1 "dma"):
  PE program: warm, E0 half-1 (e chunks {0,1} k-paced with the encT b0 DMA
  stream + ALL b0 on-chip transposes), E0 half-2 (chunks {2,3} re-sweep the
  SBUF-resident encT tiles — halves peak e-psum and starts S0 early), E1
  half-1 with late-k FILLER (pN0 groups + first c0 tile-pairs once pT0
  lands), E1 half-2, remaining c0 (covers S1's latency), pN1 (pipelined
  per-chunk behind the exps via subtile deps), c1 trailing the encN DMA
  stream in arrival order.  Softmax sum/recip are emitted late (finish_S)
  so they never block earlier-ready DVE work in the in-order queue.

DMA ring 0 (sync): z b0, encT b0 (z b1 injected after k0), encT b1,
encN b1 (GC t-tiles per transfer), encN b0 (if any) last.  Output c stored
in f16 (host upconverts) on the gpsimd SWDGE ring (c0) and the scalar
HWDGE ring (c1), with the final evac split across DVE and ACT.

Sharding: data-parallel over batch B=16 across 8 cores (2 per core).
"""

import functools
import os
import sys

import numpy as np

for _p in ("/opt/trn_rl_repo", "/root/.axon_site/_ro/trn_rl_repo"):
    if os.path.isdir(_p) and _p not in sys.path:
        sys.path.append(_p)

B, T, Q, H = 16, 2048, 64, 1024
NCORES = 8
BL = B // NCORES  # batches per core
KT = H // 128  # 8 contraction tiles for e
NT = T // 128  # 16 t-tiles
NC_CHUNK = T // 512  # 4 psum chunks along T for e
SC = 2048.0  # 2^11 scale for the zl correction channel

GE = 1  # k-tiles per encT DMA transfer (half-sweep phase_E assumes 1)
GC = int(os.environ.get("ATTN_GC", "1"))  # t-tiles per encN DMA transfer
NLOADS = tuple(
    int(x) for x in os.environ.get("ATTN_NLOADS", "0,16").split(",")
)
WARM = int(os.environ.get("ATTN_WARM", "24"))
ETBUFS = int(os.environ.get("ATTN_ETBUFS", "16"))
C0_IN_E1 = int(os.environ.get("ATTN_C0_IN_E1", "6"))  # c0 tile-pairs inside E1
FILL_K0 = int(os.environ.get("ATTN_FILL_K0", "5"))  # first E1 k-tile with filler
WF_E = int(os.environ.get("ATTN_WF_E", "0"))  # warm fills per E0 k-tile
ZDUAL = int(os.environ.get("ATTN_ZDUAL", "0"))  # zh+zl dual-channel z
ZW = 2 * Q if ZDUAL else Q  # stationary z width
C1F_FROM = int(os.environ.get("ATTN_C1F_FROM", "99"))  # first c1 tile with fills
C1F_N = int(os.environ.get("ATTN_C1F_N", "0"))  # warm fills per trailing c1 tile
COUT16 = int(os.environ.get("ATTN_COUT16", "1"))
HALFE = int(os.environ.get("ATTN_HALFE", "0"))  # split e into two half-T sweeps


@functools.lru_cache(maxsize=4)
def _build(
    loop_n: int = 1,
    nloads: tuple = NLOADS,
    c0_in_e1: int = C0_IN_E1,
    cout16: int = COUT16,
):
    import contextlib

    import concourse.mybir as mybir
    import concourse.tile as tile
    from concourse import bacc
    from concourse.bass import ts
    from concourse.masks import make_identity

    f32 = mybir.dt.float32
    f16 = mybir.dt.float16

    nc = bacc.Bacc(
        "TRN2",
        target_bir_lowering=False,
        debug=False,
        enable_asserts=False,
        num_devices=NCORES,
    )

    encT_d = nc.dram_tensor(
        "encT", [BL, KT // GE, 128, GE * T], f16, kind="ExternalInput"
    )
    n_enc_dma = sum(nloads) // GC
    if n_enc_dma:
        encN_d = nc.dram_tensor(
            "encN", [n_enc_dma, 128, GC * H], f16, kind="ExternalInput"
        )
    z_d = nc.dram_tensor("z", [BL, 128, KT, ZW], f16, kind="ExternalInput")
    c_d = nc.dram_tensor("c", [BL, Q, H], f16 if cout16 else f32, kind="ExternalOutput")

    with tile.TileContext(nc) as tc:

        with (
            tc.tile_pool(name="encT", bufs=ETBUFS) as p_encT,
            tc.tile_pool(name="encN", bufs=2) as p_encN,
            tc.tile_pool(name="z", bufs=2) as p_z,
            tc.tile_pool(name="eT", bufs=2) as p_eT,
            tc.tile_pool(name="pT", bufs=2) as p_pT,
            tc.tile_pool(name="pN", bufs=2) as p_pN,
            tc.tile_pool(name="outs", bufs=2) as p_out,
            tc.tile_pool(name="stats", bufs=12) as p_stats,
            tc.tile_pool(name="singles", bufs=1) as p_singles,
            tc.tile_pool(name="ps", bufs=7, space="PSUM") as ps,
        ):
            ident128 = p_singles.tile([128, 128], f16)
            make_identity(nc, ident128)
            ident64 = p_singles.tile([64, 64], f16)
            make_identity(nc, ident64)

            loop_ctx = (
                tc.For_i(0, loop_n, 1) if loop_n > 1 else contextlib.nullcontext()
            )
            with loop_ctx:
                z_ts, encN_sbs = [], []
                for b in range(BL):
                    z_t = p_z.tile([128, KT, ZW], f16, tag="z", name=f"z_{b}")
                    z_ts.append(z_t)
                    encN_sb = p_encN.tile(
                        [128, NT, H], f16, tag="encN", name=f"encN_{b}"
                    )
                    encN_sbs.append(encN_sb)
                # z_b0 first on the sync ring (E0 is gated on it); z_b1 is
                # injected into the ring right after encT k0 (see phase_E's
                # dma_hook) so it can't cut ahead of E0's first tile
                nc.sync.dma_start(out=z_ts[0][:], in_=z_d.ap()[0])

                def load_encN(b):
                    # host packs per-batch pieces in b order; ring-issue
                    # order is the caller's choice
                    encN_sb = encN_sbs[b]
                    base = sum(nloads[:b]) // GC
                    for i in range(nloads[b] // GC):
                        nc.sync.dma_start(
                            out=encN_sb[:, GC * i : GC * (i + 1), :],
                            in_=encN_d.ap()[base + i],
                        )

                # ---- phase helpers ----------------------------------------
                def tr_k(b, encT_g, g, k):
                    """transpose batch b's on-chip t-tiles of k-slice k from
                    the (still SBUF-resident) encT group into encN_sb[b]."""
                    encN_sb = encN_sbs[b]
                    nload = nloads[b]
                    ntr = NT - nload
                    gsz = 8
                    for gi, h0 in enumerate(range(0, ntr, gsz)):
                        hn = min(gsz, ntr - h0)
                        tp = ps.tile(
                            [128, hn, 128], f16, tag="ps", name=f"tp_{b}_{k}_{h0}"
                        )
                        for j in range(hn):
                            tt = nload + h0 + j
                            src_ap = encT_g[
                                :, g * T + tt * 128 : g * T + (tt + 1) * 128
                            ]
                            nc.tensor.matmul(
                                tp[:, j, :],
                                lhsT=src_ap,
                                rhs=ident128[:],
                                is_transpose=True,
                                start=True,
                                stop=True,
                                skip_group_check=True,
                            )
                        dst = encN_sb[:, nload + h0 : nload + h0 + hn, ts(k, 128)]
                        if (k + gi) % 2 == 0:
                            nc.vector.tensor_copy(out=dst, in_=tp[:])
                        else:
                            nc.scalar.copy(out=dst, in_=tp[:])

                def phase_E(
                    b, tr_inline=False, filler=None, wf=0, dma_hook=None, half=True
                ):
                    """e^T[b] = Zpk[b]^T @ encT[b], in TWO half-T sweeps:
                    chunks {0,1} accumulate k-paced with the DMA stream, then
                    chunks {2,3} re-sweep the SBUF-resident encT tiles.  This
                    halves peak e-psum (2 banks live instead of 4) and lets
                    chunk {0,1}'s softmax chain start one sweep early.
                    `tr_inline` emits this batch's on-chip transposes per
                    k-tile; `filler` is a list of (min_k, weight, emitter) of
                    independent PE work drained once k >= min_k (budget: one
                    heavy unit per k-tile); `wf` pads each k-tile with warm
                    transposes."""
                    z_t = z_ts[b]
                    ntr = NT - nloads[b]
                    cis = (0, 1) if half else tuple(range(NC_CHUNK))
                    e_pss = [None] * NC_CHUNK
                    for ci in cis:
                        e_pss[ci] = ps.tile(
                            [128, 512], f32, tag="ps", name=f"e_ps_{b}_{ci}"
                        )
                    encTs = []
                    fill_i = [0]

                    def drain_filler(k):
                        if not filler:
                            return
                        last = k >= KT - 1
                        heavy = 99 if last else 2
                        while fill_i[0] < len(filler):
                            min_k, weight, em = filler[fill_i[0]]
                            if not last and (min_k > k or weight > heavy):
                                break
                            em()
                            fill_i[0] += 1
                            heavy -= weight

                    for k in range(KT):
                        encT_g = p_encT.tile([128, T], f16, tag="encT")
                        nc.sync.dma_start(out=encT_g[:], in_=encT_d.ap()[b, k])
                        if k == 0 and dma_hook is not None:
                            dma_hook()
                        encTs.append(encT_g)
                        for ci in cis:
                            nc.tensor.matmul(
                                e_pss[ci][0:ZW, :],
                                lhsT=z_t[:, k, :],
                                rhs=encT_g[:, ts(ci, 512)],
                                start=(k == 0),
                                stop=(k == KT - 1),
                                skip_group_check=True,
                            )
                        if tr_inline and ntr:
                            tr_k(b, encT_g, 0, k)
                        if wf:
                            warm_fill(wf)
                        drain_filler(k)
                    return e_pss, encTs

                def phase_E2(b, e_pss, encTs):
                    """second e sweep: chunks {2,3} from SBUF-resident encT."""
                    z_t = z_ts[b]
                    for ci in (2, 3):
                        e_pss[ci] = ps.tile(
                            [128, 512], f32, tag="ps", name=f"e_ps_{b}_{ci}"
                        )
                    for k in range(KT):
                        for ci in (2, 3):
                            nc.tensor.matmul(
                                e_pss[ci][0:ZW, :],
                                lhsT=z_t[:, k, :],
                                rhs=encTs[k][:, ts(ci, 512)],
                                start=(k == 0),
                                stop=(k == KT - 1),
                                skip_group_check=True,
                            )

                def phase_S(b, e_pss):
                    """softmax stats over T; p in f16.

                    Single-channel z (default): chunk maxes on DVE read the
                    e psum directly, and the exps on ACT read the SAME psum
                    (scale/bias fused) — no eT assembly at all.  Dual-channel
                    (ZDUAL=1) assembles eT = lo/SC + hi via ACT copy + DVE
                    add per chunk (the BIR verifier allows only one PSUM
                    input per instruction and Pool cannot touch PSUM).
                    Chunk pair {0,1} is emitted first so its work pipelines
                    under the second e sweep; negm is emitted right after the
                    chunk-{2,3} maxes so exps start as early as possible.
                    """
                    m4 = p_stats.tile([64, NC_CHUNK], f32, tag="m4", name=f"m4_{b}")
                    negm = p_stats.tile([64, 1], f32, tag="negm", name=f"negm_{b}")
                    eT = None
                    if ZDUAL:
                        eT = p_eT.tile([64, T], f32, tag="eT", name=f"eT_{b}")

                    def emit_maxes(cis):
                        for ci in cis:
                            nc.vector.reduce_max(
                                out=m4[:, ci : ci + 1],
                                in_=e_pss[ci][0:64, :],
                                axis=mybir.AxisListType.X,
                            )

                    def emit_assembly(cis):
                        for ci in cis:
                            nc.scalar.activation(
                                out=eT[:, ts(ci, 512)],
                                in_=e_pss[ci][64:128, :],
                                func=mybir.ActivationFunctionType.Copy,
                                bias=0.0,
                                scale=1.0 / SC,
                            )
                        for ci in cis:
                            nc.vector.tensor_add(
                                eT[:, ts(ci, 512)],
                                eT[:, ts(ci, 512)],
                                e_pss[ci][0:64, :],
                            )

                    emit_maxes((0, 1))
                    if ZDUAL:
                        emit_assembly((0, 1))
                    emit_maxes((2, 3))
                    nc.vector.reduce_max(
                        out=negm[:], in_=m4[:], axis=mybir.AxisListType.X, negate=True
                    )
                    if ZDUAL:
                        emit_assembly((2, 3))
                    pT = p_pT.tile([64, T], f16, tag="pT", name=f"pT_{b}")
                    s4 = p_stats.tile([64, NC_CHUNK], f32, tag="s4", name=f"s4_{b}")
                    for ci in range(NC_CHUNK):
                        nc.scalar.activation(
                            out=pT[:, ts(ci, 512)],
                            in_=eT[:, ts(ci, 512)] if ZDUAL else e_pss[ci][0:64, :],
                            func=mybir.ActivationFunctionType.Exp,
                            bias=negm[:],
                            scale=1.0,
                            accum_out=s4[:, ci : ci + 1],
                        )
                    return pT, s4

                def finish_S(b, s4):
                    """sum+reciprocal, emitted late so these DVE ops never
                    block earlier-ready DVE work in the in-order queue."""
                    s_sum = p_stats.tile([64, 1], f32, tag="s", name=f"s_{b}")
                    nc.vector.reduce_sum(
                        out=s_sum[:], in_=s4[:], axis=mybir.AxisListType.X
                    )
                    r = p_stats.tile([64, 1], f32, tag="r", name=f"r_{b}")
                    nc.vector.reciprocal(out=r[:], in_=s_sum[:])
                    return r

                def make_pN(b, pT):
                    """transpose pT [64,T] into pN [128, NT, Q]; returns a list
                    of 4 emitters (4 t-tiles each) usable as PE filler."""
                    pN = p_pN.tile([128, NT, Q], f16, tag="pN", name=f"pN_{b}")

                    def emit(tg):
                        def go():
                            trp = ps.tile(
                                [128, 4, Q], f16, tag="ps", name=f"trp_{b}_{tg}",
                            )
                            for j in range(4):
                                tt = tg * 4 + j
                                nc.tensor.matmul(
                                    trp[:, j, :],
                                    lhsT=pT[:, ts(tt, 128)],
                                    rhs=ident64[:],
                                    is_transpose=True,
                                    start=True,
                                    stop=True,
                                    skip_group_check=True,
                                )
                            nc.vector.tensor_copy(
                                out=pN[:, tg * 4 : (tg + 1) * 4, :], in_=trp[:]
                            )

                        return go

                    return pN, [emit(tg) for tg in range(NT // 4)]

                c_pss, outs = {}, {}

                def c_tile(b, pN, tt, i):
                    """accumulate c[b] += pN[tt]^T @ encN[b][tt] (both H halves
                    into one psum bank via tile_position)."""
                    if b not in c_pss:
                        c_pss[b] = ps.tile(
                            [128, 512], f32, tag="ps", name=f"c_{b}"
                        )
                    c_ps = c_pss[b]
                    encN_sb = encN_sbs[b]
                    nc.tensor.matmul(
                        c_ps[0:64, :],
                        lhsT=pN[:, tt, :],
                        rhs=encN_sb[:, tt, 0:512],
                        start=(i == 0),
                        stop=(i == NT - 1),
                        skip_group_check=True,
                    )
                    nc.tensor.matmul(
                        c_ps[64:128, :],
                        lhsT=pN[:, tt, :],
                        rhs=encN_sb[:, tt, 512:1024],
                        start=(i == 0),
                        stop=(i == NT - 1),
                        tile_position=(0, 64),
                        skip_group_check=True,
                    )

                def c_finish(b, r, ring, split=False):
                    c_ps = c_pss[b]
                    out_t = p_out.tile(
                        [64, H], f16 if cout16 else f32, tag="out", name=f"out_{b}"
                    )
                    nc.vector.tensor_scalar_mul(out_t[:, 0:512], c_ps[0:64, :], r[:])
                    if split:
                        # halves in parallel on DVE and ACT (tail-critical)
                        nc.scalar.activation(
                            out=out_t[:, 512:1024],
                            in_=c_ps[64:128, :],
                            func=mybir.ActivationFunctionType.Copy,
                            bias=0.0,
                            scale=r[:],
                        )
                    else:
                        nc.vector.tensor_scalar_mul(
                            out_t[:, 512:1024], c_ps[64:128, :], r[:]
                        )
                    ring.dma_start(out=c_d.ap()[b], in_=out_t[:])

                # PE warm-up / p-state filler: data-independent transposes
                # keep the tensor engine busy so it holds its max p-state
                # own tag: lives the whole iteration, must not hold a slot
                # of the main psum ring
                warm_ps = ps.tile([128, 128], f16, tag="warm", bufs=1, name="warm")

                def warm_fill(n):
                    for _ in range(n):
                        nc.tensor.matmul(
                            warm_ps[:],
                            lhsT=ident128[:],
                            rhs=ident128[:],
                            is_transpose=True,
                            start=True,
                            stop=True,
                            skip_group_check=True,
                        )

                warm_fill(WARM)

                # ---- schedule ----------------------------------------------
                # PE program: warm, E0 half-1 (e chunks 0,1 + b0 transposes,
                # paced with the encT b0 DMA stream), E0 half-2 (chunks 2,3
                # from SBUF), E1 half-1 (+ filler: pN0 groups and first c0
                # tiles in the late k-slots once pT0 exists), E1 half-2,
                # remaining c0, pN1, c1 trailing the encN DMA stream.
                b0, b1 = 0, 1
                e_pss0, encTs0 = phase_E(
                    b0,
                    tr_inline=True,
                    wf=WF_E,
                    dma_hook=lambda: nc.sync.dma_start(
                        out=z_ts[1][:], in_=z_d.ap()[1]
                    ),
                    half=bool(HALFE),
                )
                if HALFE:
                    phase_E2(b0, e_pss0, encTs0)
                pT0, s4_0 = phase_S(b0, e_pss0)
                pN0, pn0_emitters = make_pN(b0, pT0)

                order0 = list(range(nloads[b0], NT)) + list(range(nloads[b0]))
                c0_sched = [
                    (lambda tt=tt, i=i: c_tile(b0, pN0, tt, i))
                    for i, tt in enumerate(order0)
                ]
                n_fill = min(c0_in_e1, 2 * (KT - FILL_K0))
                filler = []
                for gi in range(4):
                    filler.append((FILL_K0 + gi, 0, pn0_emitters[gi]))
                for j in range(n_fill):
                    filler.append((FILL_K0 + j // 2, 1, c0_sched[j]))
                filler.sort(key=lambda t: t[0])

                e_pss1, encTs1 = phase_E(b1, filler=filler, half=bool(HALFE))
                load_encN(b1)
                # b0's DMA'd tiles (if any) ride LAST on the ring: they feed
                # c0's final matmuls without delaying c1's encN stream
                load_encN(b0)
                if HALFE:
                    phase_E2(b1, e_pss1, encTs1)
                pT1, s4_1 = phase_S(b1, e_pss1)
                # remaining c0 tiles run on PE during S1's ACT/DVE/Pool chain
                done0 = n_fill
                while done0 < NT:
                    c0_sched[done0]()
                    done0 += 1
                r0 = finish_S(b0, s4_0)
                c_finish(b0, r0, nc.gpsimd)
                # pn1 group ci depends only on exp chunk ci (subtile deps)
                pN1, pn1_emitters = make_pN(b1, pT1)
                for em in pn1_emitters:
                    em()
                # c1 tiles trail the encN b1 DMA stream in arrival order;
                # optional warm pads hold the p-state through the trailing
                # arrivals (no-op in the timeline model, HW knob)
                order1 = list(range(nloads[b1], NT)) + list(range(nloads[b1]))
                for i, tt in enumerate(order1):
                    c_tile(b1, pN1, tt, i)
                    if i >= C1F_FROM and i < NT - 1:
                        warm_fill(C1F_N)
                r1 = finish_S(b1, s4_1)
                c_finish(b1, r1, nc.scalar, split=True)

    nc.compile()
    return nc


def _host_prep(h_enc, h_dec, W_psi, b_psi, W_phi, b_phi, nloads: tuple = NLOADS):
    h_enc = np.asarray(h_enc, dtype=np.float32)
    h_dec = np.asarray(h_dec, dtype=np.float32)
    W_psi = np.asarray(W_psi, dtype=np.float64)
    W_phi = np.asarray(W_phi, dtype=np.float64)
    b_phi = np.asarray(b_phi, dtype=np.float64)

    # M = W_psi^T @ W_phi [H, H];  u = W_psi^T @ b_phi [H]
    M = W_psi.T @ W_phi
    u = W_psi.T @ b_phi
    # Z[b, h, q] = sum_k M[h, k] * h_dec[q, b, k] + u[h]
    dec_r = h_dec.astype(np.float64).transpose(2, 1, 0).reshape(H, B * Q)
    Z = (M @ dec_r).reshape(H, B, Q).transpose(1, 0, 2) + u[None, :, None]
    Z = np.ascontiguousarray(Z, dtype=np.float32)  # [B, H, Q]

    def tile_i(x, g):  # [B, G*g*128, W] -> [B, G, 128, g*W] interleaved
        Bn, R, W = x.shape
        G = R // (g * 128)
        return np.ascontiguousarray(
            x.reshape(Bn, G, g, 128, W).transpose(0, 1, 3, 2, 4).reshape(
                Bn, G, 128, g * W
            )
        )

    encT = np.ascontiguousarray(h_enc.transpose(0, 2, 1))  # [B, H, T] fp32
    arrays = {"encT": tile_i(encT.astype(np.float16), GE)}

    zh = Z.astype(np.float16)
    if ZDUAL:
        zl = ((Z - zh.astype(np.float32)) * SC).astype(np.float16)
        zpk = np.concatenate([zh, zl], axis=2)  # [B, H, 2Q]
    else:
        zpk = zh
    arrays["z"] = np.ascontiguousarray(
        zpk.reshape(B, KT, 128, ZW).transpose(0, 2, 1, 3)
    )  # [B, 128, KT, ZW]

    if sum(nloads):
        encN16 = h_enc.astype(np.float16)  # [B, T, H]
        pieces = []
        for core in range(NCORES):
            for bl, nload in enumerate(nloads):
                if nload:
                    bglob = core * BL + bl
                    pieces.append(
                        tile_i(encN16[bglob : bglob + 1, : nload * 128, :], GC)[0]
                    )
        # [NCORES, sum(nloads)//GC, 128, GC*H]
        arrays["encN"] = np.ascontiguousarray(
            np.concatenate(pieces, 0).reshape(NCORES, -1, 128, GC * H)
        )
    return arrays


def _in_maps(arrays):
    maps = []
    for i in range(NCORES):
        m = {}
        for k, v in arrays.items():
            if k == "encN":
                m[k] = v[i]
            else:
                m[k] = v[i * BL : (i + 1) * BL]
        maps.append(m)
    return maps


def kernel(h_enc, h_dec, W_psi, b_psi, W_phi, b_phi):
    from concourse.bass_utils import run_bass_kernel_spmd

    arrays = _host_prep(h_enc, h_dec, W_psi, b_psi, W_phi, b_phi)
    nc = _build()
    res = run_bass_kernel_spmd(nc, _in_maps(arrays), core_ids=list(range(NCORES)))
    out = np.concatenate([res.results[i]["c"] for i in range(NCORES)], axis=0)
    return np.ascontiguousarray(out, dtype=np.float32)
